# revision 14
# baseline (speedup 1.0000x reference)
"""Class-aware TCR loss on 8 Trainium2 NeuronCores.

Math (see the reference): rows of `feat` are L2-normalized; each point
belongs to exactly one of G = B*C = 16 disjoint (batch, class) groups;
per group we need gram_g = Zn_g^T @ Zn_g (D x D), counts, then a tiny
logdet/deficit reduction over 16 matrices.

Strategy
 - Host preprocessing: L2-normalize rows (one fp32 numpy pass), bucket
   rows by group id (integer argsort), pad each group to a fixed S rows,
   hand 2 groups to each of the 8 cores as a contiguous bf16 array
   pre-arranged so every device DMA is fully contiguous.
 - Device (per core): pure Gram compute on the tensor engine — for each
   128-row tile accumulate gram = Z^T Z into PSUM in fp32.  Because the
   Gram is symmetric, only the upper block row is computed: for D=256
   split into chunks [0,1], compute blocks [0,0],[0,1] (one 256-wide
   matmul) and [1,1] (one 128-wide matmul); the host mirrors [1,0].
 - Host: assemble the 16 Grams in float64, slogdet, deficits, loss.

Pad rows are exactly zero and contribute nothing to the Grams.
"""

import numpy as np
import ml_dtypes

# ---- problem constants (hardcoded per the task contract) ----
N = 65536
D = 256
C = 8
B = 2
G = B * C  # 16 groups
EPS = 0.2
LAMBDA_TCR = 0.05
LOSS_WEIGHT = 1.0
MIN_SAMPLES = 10

N_CORES = 8
GROUPS_PER_CORE = G // N_CORES  # 2
S = 4096                        # rows per group on device (32 tiles of 128);
                                # overflow rows are handled on the host
TILES_PER_GROUP = S // 128      # 32
TILES_PER_CORE = TILES_PER_GROUP * GROUPS_PER_CORE  # 64
SUPER = 16                      # row-tiles per DMA super-tile
N_SUPER = TILES_PER_CORE // SUPER  # 4
GCOLS = 2 * (D + 128)           # output cols: 2 groups x (256 + 128)
# device input dtype: bf16.  (fp8 was measured slower on this hardware:
# e4m3 20.5us/iter, e3m4 38us/iter vs bf16 ~17us/iter — no FWL/DoubleRow
# path in this toolchain, so fp8 only shrinks DMA, and the kernel is
# PE-bound.)  Gram accumulates in fp32 PSUM.
BF16 = ml_dtypes.bfloat16
XDT_NAME = "bfloat16" 

_COMPILED = None   # cached Bass module so repeat kernel() calls skip tracing
TRACE = False
LAST_RESULTS = None  # BassKernelResults of the most recent device run


def _build_nc():
    import concourse.bacc as bacc
    import concourse.mybir as mybir
    from concourse.tile import TileContext

    nc = bacc.Bacc("TRN2", target_bir_lowering=False)
    # x: 9 super-tiles of [128 partitions, 8 row-tiles * 256 features]
    x_dram = nc.dram_tensor(
        "x", [N_SUPER, 128, SUPER * D], getattr(mybir.dt, XDT_NAME),
        kind="ExternalInput"
    )
    # per group j: cols [j*384, j*384+256) = blocks [0,0]+[0,1],
    #              cols [j*384+256, j*384+384) = block [1,1]
    g_dram = nc.dram_tensor(
        "gram", [128, GCOLS], mybir.dt.float32, kind="ExternalOutput"
    )

    f32 = mybir.dt.float32
    xdt = getattr(mybir.dt, XDT_NAME)

    with TileContext(nc) as tc:
        with (
            tc.tile_pool(name="io", bufs=3) as io_pool,
            tc.tile_pool(name="out", bufs=1) as out_pool,
            tc.tile_pool(name="psum", bufs=1, space="PSUM") as psum_pool,
        ):
            ps0 = [
                psum_pool.tile([128, D], f32, name=f"ps0_{j}", tag=f"ps0_{j}")
                for j in range(GROUPS_PER_CORE)
            ]
            ps1 = [
                psum_pool.tile([128, 128], f32, name=f"ps1_{j}", tag=f"ps1_{j}")
                for j in range(GROUPS_PER_CORE)
            ]

            for u in range(N_SUPER):
                xt = io_pool.tile([128, SUPER * D], xdt, tag="xt", name="xt")
                nc.sync.dma_start(out=xt, in_=x_dram[u])

                for s in range(SUPER):
                    t = u * SUPER + s  # global row-tile index
                    grp = t // TILES_PER_GROUP
                    tt = t % TILES_PER_GROUP
                    start = tt == 0
                    stop = tt == TILES_PER_GROUP - 1
                    lo = s * D
                    # blocks [0,0] and [0,1]: chunk-0 stationary, full moving
                    nc.tensor.matmul(
                        ps0[grp],
                        xt[:, lo : lo + 128],
                        xt[:, lo : lo + D],
                        start=start,
                        stop=stop,
                    )
                    # block [1,1]: chunk-1 stationary, chunk-1 moving
                    nc.tensor.matmul(
                        ps1[grp],
                        xt[:, lo + 128 : lo + D],
                        xt[:, lo + 128 : lo + D],
                        start=start,
                        stop=stop,
                    )

            gout = out_pool.tile([128, GCOLS], f32, name="gout")
            for j in range(GROUPS_PER_CORE):
                base = j * (D + 128)
                nc.scalar.copy(gout[:, base : base + D], ps0[j])
                nc.scalar.copy(gout[:, base + D : base + D + 128], ps1[j])
            nc.sync.dma_start(out=g_dram[:, :], in_=gout)

    nc.compile()
    return nc


def _get_compiled():
    global _COMPILED
    if _COMPILED is None:
        _COMPILED = _build_nc()
    return _COMPILED


def _shard_inputs(zn, gid):
    """Bucket normalized rows by group, pad to S per group, arrange per
    core so the device sees fully contiguous DMA super-tiles.

    Returns (in_maps, extras) where extras[g] is a (k, D) float32 array of
    overflow rows (only if a group exceeds S; statistically impossible for
    the target distribution, kept for correctness)."""
    order = np.argsort(gid, kind="stable")
    sorted_zn = zn[order]
    counts = np.bincount(gid, minlength=G)
    offs = np.zeros(G + 1, dtype=np.int64)
    np.cumsum(counts, out=offs[1:])

    extras = {}
    x_all = np.zeros((G, S, D), dtype=BF16)
    for g in range(G):
        rows = sorted_zn[offs[g] : offs[g + 1]]
        if rows.shape[0] > S:
            extras[g] = rows[S:].copy()
            rows = rows[:S]
        x_all[g, : rows.shape[0]] = rows.astype(BF16)

    in_maps = []
    for core in range(N_CORES):
        xc = x_all[GROUPS_PER_CORE * core : GROUPS_PER_CORE * (core + 1)]
        # (2, S, D) -> (9216, 256) -> (9, 8, 128, 256) -> (9, 128, 8*256)
        xc = xc.reshape(N_SUPER, SUPER, 128, D).transpose(0, 2, 1, 3)
        xc = np.ascontiguousarray(xc).reshape(N_SUPER, 128, SUPER * D)
        in_maps.append({"x": xc})
    return in_maps, extras


def kernel(pred=None, target=None, feat=None, batch=None):
    global LAST_RESULTS
    from concourse.bass_utils import run_bass_kernel_spmd

    feat = np.asarray(feat, dtype=np.float32)
    target = np.asarray(target).astype(np.int64)
    batch = np.asarray(batch).astype(np.int64)

    gid = (batch * C + np.clip(target, 0, C - 1)).astype(np.int64)
    counts = np.bincount(gid, minlength=G).astype(np.float64)

    # F.normalize(p=2, dim=1): x / max(||x||, 1e-12)
    norms = np.sqrt(np.einsum("ij,ij->i", feat, feat, dtype=np.float32))
    zn = feat * (1.0 / np.maximum(norms, 1e-12))[:, None]

    in_maps, extras = _shard_inputs(zn, gid)

    nc = _get_compiled()
    res = run_bass_kernel_spmd(
        nc, in_maps, core_ids=list(range(N_CORES)), trace=TRACE
    )
    LAST_RESULTS = res

    grams = np.empty((G, D, D), dtype=np.float64)
    for core in range(N_CORES):
        out = res.results[core]["gram"]  # (128, GCOLS) f32
        for j in range(GROUPS_PER_CORE):
            g = GROUPS_PER_CORE * core + j
            base = j * (D + 128)
            c0 = out[:, base : base + D].astype(np.float64)       # [0,0]+[0,1]
            c1 = out[:, base + D : base + D + 128].astype(np.float64)  # [1,1]
            grams[g, :128, :] = c0
            grams[g, 128:, :128] = c0[:, 128:].T
            grams[g, 128:, 128:] = c1

    # overflow rows (host, float64) -- statistically never taken
    for g, rows in extras.items():
        zr = rows.astype(np.float64)
        grams[g] += zr.T @ zr

    # ---- tiny logdet / deficit reduction (float64 on host) ----
    min_tcr = 0.5 * np.log(float(D))
    log_diag = np.log(1.0 + 1e-6 + 1e-12)
    eye = np.eye(D, dtype=np.float64)

    deficits = np.zeros(G, dtype=np.float64)
    for g in range(G):
        n = counts[g]
        nn = max(n, 1.0)
        a = D / (nn * EPS**2)
        M = a * grams[g] + (1.0 + 1e-6) * eye
        sign, logabsdet = np.linalg.slogdet(M)
        logdet = logabsdet + (nn - D) * log_diag
        tcr = 0.5 * logdet
        deficits[g] = max(min_tcr - tcr, 0.0)

    valid = (counts >= MIN_SAMPLES).astype(np.float64)
    per_b_sum = (deficits * valid).reshape(B, C).sum(axis=1)
    per_b_cnt = valid.reshape(B, C).sum(axis=1)
    per_batch = np.where(
        per_b_cnt > 0, per_b_sum / np.maximum(per_b_cnt, 1.0), 0.0
    )
    avg = per_batch.mean()
    loss = LOSS_WEIGHT * LAMBDA_TCR * avg
    return np.asarray(loss, dtype=np.float32)


# revision 15
# speedup vs baseline: 1.1642x; 1.1642x over previous
"""Class-aware TCR loss on 8 Trainium2 NeuronCores.

Math (see the reference): rows of `feat` are L2-normalized; each point
belongs to exactly one of G = B*C = 16 disjoint (batch, class) groups;
per group we need gram_g = Zn_g^T @ Zn_g (D x D), counts, then a tiny
logdet/deficit reduction over 16 matrices.

Strategy
 - Host preprocessing: L2-normalize rows (one fp32 numpy pass), bucket
   rows by group id (integer argsort), pad each group to a fixed S rows,
   hand 2 groups to each of the 8 cores as a contiguous bf16 array
   pre-arranged so every device DMA is fully contiguous.
 - Device (per core): pure Gram compute on the tensor engine — for each
   128-row tile accumulate gram = Z^T Z into PSUM in fp32.  Because the
   Gram is symmetric, only the upper block row is computed: for D=256
   split into chunks [0,1], compute blocks [0,0],[0,1] (one 256-wide
   matmul) and [1,1] (one 128-wide matmul); the host mirrors [1,0].
 - Host: assemble the 16 Grams in float64, slogdet, deficits, loss.

Pad rows are exactly zero and contribute nothing to the Grams.
"""

import numpy as np
import ml_dtypes

# ---- problem constants (hardcoded per the task contract) ----
N = 65536
D = 256
C = 8
B = 2
G = B * C  # 16 groups
EPS = 0.2
LAMBDA_TCR = 0.05
LOSS_WEIGHT = 1.0
MIN_SAMPLES = 10

N_CORES = 8
GROUPS_PER_CORE = G // N_CORES  # 2
S = 4096                        # rows per group on device (32 tiles of 128);
                                # overflow rows are handled on the host
TILES_PER_GROUP = S // 128      # 32
TILES_PER_CORE = TILES_PER_GROUP * GROUPS_PER_CORE  # 64
SUPER = 16                      # row-tiles per DMA super-tile
N_SUPER = TILES_PER_CORE // SUPER  # 4
GCOLS = 2 * (D + 128)           # output cols: 2 groups x (256 + 128)
# device input dtype: fp8 e4m3.  Measured per-iteration HW time (For_i
# slope at K=32776): e4m3 20.5us < bf16 23.9us < e3m4 38us.  fp8 halves
# DMA traffic; the Gram accumulates exactly in fp32 PSUM, and the deficit
# margin (~400 vs min_tcr) makes 3-mantissa-bit inputs numerically safe.
BF16 = ml_dtypes.float8_e4m3
XDT_NAME = "float8e4" 

_COMPILED = None   # cached Bass module so repeat kernel() calls skip tracing
TRACE = False
LAST_RESULTS = None  # BassKernelResults of the most recent device run


def _build_nc():
    import concourse.bacc as bacc
    import concourse.mybir as mybir
    from concourse.tile import TileContext

    nc = bacc.Bacc("TRN2", target_bir_lowering=False)
    # x: 9 super-tiles of [128 partitions, 8 row-tiles * 256 features]
    x_dram = nc.dram_tensor(
        "x", [N_SUPER, 128, SUPER * D], getattr(mybir.dt, XDT_NAME),
        kind="ExternalInput"
    )
    # per group j: cols [j*384, j*384+256) = blocks [0,0]+[0,1],
    #              cols [j*384+256, j*384+384) = block [1,1]
    g_dram = nc.dram_tensor(
        "gram", [128, GCOLS], mybir.dt.float32, kind="ExternalOutput"
    )

    f32 = mybir.dt.float32
    xdt = getattr(mybir.dt, XDT_NAME)

    with TileContext(nc) as tc:
        with (
            tc.tile_pool(name="io", bufs=4) as io_pool,
            tc.tile_pool(name="out", bufs=1) as out_pool,
            tc.tile_pool(name="psum", bufs=1, space="PSUM") as psum_pool,
        ):
            ps0 = [
                psum_pool.tile([128, D], f32, name=f"ps0_{j}", tag=f"ps0_{j}")
                for j in range(GROUPS_PER_CORE)
            ]
            ps1 = [
                psum_pool.tile([128, 128], f32, name=f"ps1_{j}", tag=f"ps1_{j}")
                for j in range(GROUPS_PER_CORE)
            ]

            for u in range(N_SUPER):
                xt = io_pool.tile([128, SUPER * D], xdt, tag="xt", name="xt")
                nc.sync.dma_start(out=xt, in_=x_dram[u])

                for s in range(SUPER):
                    t = u * SUPER + s  # global row-tile index
                    grp = t // TILES_PER_GROUP
                    tt = t % TILES_PER_GROUP
                    start = tt == 0
                    stop = tt == TILES_PER_GROUP - 1
                    lo = s * D
                    # blocks [0,0] and [0,1]: chunk-0 stationary, full moving
                    nc.tensor.matmul(
                        ps0[grp],
                        xt[:, lo : lo + 128],
                        xt[:, lo : lo + D],
                        start=start,
                        stop=stop,
                    )
                    # block [1,1]: chunk-1 stationary, chunk-1 moving
                    nc.tensor.matmul(
                        ps1[grp],
                        xt[:, lo + 128 : lo + D],
                        xt[:, lo + 128 : lo + D],
                        start=start,
                        stop=stop,
                    )

            gout = out_pool.tile([128, GCOLS], f32, name="gout")
            for j in range(GROUPS_PER_CORE):
                base = j * (D + 128)
                nc.scalar.copy(gout[:, base : base + D], ps0[j])
                nc.scalar.copy(gout[:, base + D : base + D + 128], ps1[j])
            nc.sync.dma_start(out=g_dram[:, :], in_=gout)

    nc.compile()
    return nc


def _get_compiled():
    global _COMPILED
    if _COMPILED is None:
        _COMPILED = _build_nc()
    return _COMPILED


def _shard_inputs(zn, gid):
    """Bucket normalized rows by group, pad to S per group, arrange per
    core so the device sees fully contiguous DMA super-tiles.

    Returns (in_maps, extras) where extras[g] is a (k, D) float32 array of
    overflow rows (only if a group exceeds S; statistically impossible for
    the target distribution, kept for correctness)."""
    order = np.argsort(gid, kind="stable")
    sorted_zn = zn[order]
    counts = np.bincount(gid, minlength=G)
    offs = np.zeros(G + 1, dtype=np.int64)
    np.cumsum(counts, out=offs[1:])

    extras = {}
    x_all = np.zeros((G, S, D), dtype=BF16)
    for g in range(G):
        rows = sorted_zn[offs[g] : offs[g + 1]]
        if rows.shape[0] > S:
            extras[g] = rows[S:].copy()
            rows = rows[:S]
        x_all[g, : rows.shape[0]] = rows.astype(BF16)

    in_maps = []
    for core in range(N_CORES):
        xc = x_all[GROUPS_PER_CORE * core : GROUPS_PER_CORE * (core + 1)]
        # (2, S, D) -> (9216, 256) -> (9, 8, 128, 256) -> (9, 128, 8*256)
        xc = xc.reshape(N_SUPER, SUPER, 128, D).transpose(0, 2, 1, 3)
        xc = np.ascontiguousarray(xc).reshape(N_SUPER, 128, SUPER * D)
        in_maps.append({"x": xc})
    return in_maps, extras


def kernel(pred=None, target=None, feat=None, batch=None):
    global LAST_RESULTS
    from concourse.bass_utils import run_bass_kernel_spmd

    feat = np.asarray(feat, dtype=np.float32)
    target = np.asarray(target).astype(np.int64)
    batch = np.asarray(batch).astype(np.int64)

    gid = (batch * C + np.clip(target, 0, C - 1)).astype(np.int64)
    counts = np.bincount(gid, minlength=G).astype(np.float64)

    # F.normalize(p=2, dim=1): x / max(||x||, 1e-12)
    norms = np.sqrt(np.einsum("ij,ij->i", feat, feat, dtype=np.float32))
    zn = feat * (1.0 / np.maximum(norms, 1e-12))[:, None]

    in_maps, extras = _shard_inputs(zn, gid)

    nc = _get_compiled()
    res = run_bass_kernel_spmd(
        nc, in_maps, core_ids=list(range(N_CORES)), trace=TRACE
    )
    LAST_RESULTS = res

    grams = np.empty((G, D, D), dtype=np.float64)
    for core in range(N_CORES):
        out = res.results[core]["gram"]  # (128, GCOLS) f32
        for j in range(GROUPS_PER_CORE):
            g = GROUPS_PER_CORE * core + j
            base = j * (D + 128)
            c0 = out[:, base : base + D].astype(np.float64)       # [0,0]+[0,1]
            c1 = out[:, base + D : base + D + 128].astype(np.float64)  # [1,1]
            grams[g, :128, :] = c0
            grams[g, 128:, :128] = c0[:, 128:].T
            grams[g, 128:, 128:] = c1

    # overflow rows (host, float64) -- statistically never taken
    for g, rows in extras.items():
        zr = rows.astype(np.float64)
        grams[g] += zr.T @ zr

    # ---- tiny logdet / deficit reduction (float64 on host) ----
    min_tcr = 0.5 * np.log(float(D))
    log_diag = np.log(1.0 + 1e-6 + 1e-12)
    eye = np.eye(D, dtype=np.float64)

    deficits = np.zeros(G, dtype=np.float64)
    for g in range(G):
        n = counts[g]
        nn = max(n, 1.0)
        a = D / (nn * EPS**2)
        M = a * grams[g] + (1.0 + 1e-6) * eye
        sign, logabsdet = np.linalg.slogdet(M)
        logdet = logabsdet + (nn - D) * log_diag
        tcr = 0.5 * logdet
        deficits[g] = max(min_tcr - tcr, 0.0)

    valid = (counts >= MIN_SAMPLES).astype(np.float64)
    per_b_sum = (deficits * valid).reshape(B, C).sum(axis=1)
    per_b_cnt = valid.reshape(B, C).sum(axis=1)
    per_batch = np.where(
        per_b_cnt > 0, per_b_sum / np.maximum(per_b_cnt, 1.0), 0.0
    )
    avg = per_batch.mean()
    loss = LOSS_WEIGHT * LAMBDA_TCR * avg
    return np.asarray(loss, dtype=np.float32)
